# revision 2
# baseline (speedup 1.0000x reference)
"""DistogramHead Trainium2 kernel.

Computes out[b, i, j] = relu(0.5*(s_i[b,i] + s_j[b,j]) + b_out) where
  s_i = (x @ w_i + b_i) @ w_out  = x @ (w_i @ w_out) + b_i @ w_out
  s_j = (x @ w_j + b_j) @ w_out  = x @ (w_j @ w_out) + b_j @ w_out

Shapes: x (4, 4096, 256) f32 -> out (4, 4096, 4096) f32 (256 MB).
Memory-bound on the output write.

Sharding over 8 cores: core c handles batch b = c//2, row half r = c%2,
producing the contiguous slab out[b, r*2048:(r+1)*2048, :] (32 MB/core).
Each core receives the full batch x[b] (for s_j over all 4096 columns)
plus its own 2048 rows again (for s_i) - this avoids any per-core
control flow so one SPMD program serves all 8 cores.

On-device pipeline per core:
  1. v_i, v_j = w @ w_out folded on-device via DVE multiply + reduce.
  2. s_j for all 32 row-blocks and s_i for own 16 row-blocks via DVE
     (x_block * v_bcast, reduce over d=256).
  3. s_j column tiles -> PE transpose -> DRAM bounce -> partition-stride-0
     broadcast DMA -> Rb (128, 4096): every partition holds the s_j row.
  4. bias columns A = 0.5*s_i + (0.5*(c_i+c_j) + b_out)  (c = b @ w_out).
  5. 16 output tiles: ACT relu(0.5*Rb + A[:, t]) -> 2 MB DMA store.
"""

import numpy as np

B = 4
L = 4096
D = 256
H = 128
P = 128
NCORES = 8
ROWS_PER_CORE = L // 2          # 2048
NBLK_ALL = L // P               # 32
NBLK_OWN = ROWS_PER_CORE // P   # 16

_PROGRAM = None


def _build_program():
    import concourse.bacc as bacc
    import concourse.bass as bass
    import concourse.tile as tile
    from concourse import mybir

    f32 = mybir.dt.float32
    nc = bacc.Bacc(None)

    xall = nc.dram_tensor("xall", [L, D], f32, kind="ExternalInput")
    xown = nc.dram_tensor("xown", [ROWS_PER_CORE, D], f32, kind="ExternalInput")
    wi = nc.dram_tensor("wi", [D, H], f32, kind="ExternalInput")
    wj = nc.dram_tensor("wj", [D, H], f32, kind="ExternalInput")
    bi = nc.dram_tensor("bi", [1, H], f32, kind="ExternalInput")
    bj = nc.dram_tensor("bj", [1, H], f32, kind="ExternalInput")
    wout = nc.dram_tensor("wout", [1, H], f32, kind="ExternalInput")
    bout = nc.dram_tensor("bout", [1, 1], f32, kind="ExternalInput")
    ident = nc.dram_tensor("ident", [P, P], f32, kind="ExternalInput")
    out = nc.dram_tensor("out", [ROWS_PER_CORE, L], f32, kind="ExternalOutput")

    def dram_bcast(ap, nparts):
        """AP reading the same DRAM region once per partition (stride 0)."""
        return bass.AP(tensor=ap.tensor, offset=ap.offset, ap=[[0, nparts]] + list(ap.ap))

    def mul_reduce(junk_pool, tag, in0, in1, accum, shape):
        junk = junk_pool.tile(shape, f32, tag=tag)
        nc.vector.tensor_mul(junk[:], in0, in1)
        nc.vector.reduce_sum(accum, junk[:], axis=mybir.AxisListType.X)

    with tile.TileContext(nc) as tc:
        with (
            tc.tile_pool(name="persist", bufs=1) as persist,
            tc.tile_pool(name="junkp", bufs=3) as junkp,
            tc.tile_pool(name="outp", bufs=3) as outp,
            tc.tile_pool(name="psum", bufs=2, space="PSUM") as psum,
            tc.tile_pool(name="dram", bufs=1, space="DRAM") as drampool,
        ):
            # ---- constants ----
            ident_sb = persist.tile([P, P], f32)
            nc.sync.dma_start(out=ident_sb[:], in_=ident[:, :])

            wout_bc = persist.tile([P, H], f32)
            nc.gpsimd.dma_start(out=wout_bc[:], in_=dram_bcast(wout[0, :], P))
            bi_bc = persist.tile([P, H], f32)
            nc.gpsimd.dma_start(out=bi_bc[:], in_=dram_bcast(bi[0, :], P))
            bj_bc = persist.tile([P, H], f32)
            nc.gpsimd.dma_start(out=bj_bc[:], in_=dram_bcast(bj[0, :], P))
            bout_col = persist.tile([P, 1], f32)
            nc.gpsimd.dma_start(out=bout_col[:], in_=dram_bcast(bout[0, :], P))

            # w_i/w_j natural layout: partition = d (two chunks of 128), free = h
            wi_sb = persist.tile([P, 2, H], f32)
            nc.sync.dma_start(out=wi_sb[:], in_=wi.rearrange("(c p) h -> p c h", p=P))
            wj_sb = persist.tile([P, 2, H], f32)
            nc.sync.dma_start(out=wj_sb[:], in_=wj.rearrange("(c p) h -> p c h", p=P))

            # v columns: v[d] = sum_h w[d, h] * w_out[h]; Vcols[:, 0:2]=v_i, 2:4=v_j
            vcols = persist.tile([P, 4], f32)
            for idx, (w_sb, col) in enumerate([(wi_sb, 0), (wi_sb, 1), (wj_sb, 0), (wj_sb, 1)]):
                mul_reduce(junkp, "junk", w_sb[:, col, :], wout_bc[:],
                           vcols[:, idx : idx + 1], [P, H])

            # c_i, c_j scalars (replicated on all partitions)
            ci_col = persist.tile([P, 1], f32)
            mul_reduce(junkp, "junk", bi_bc[:], wout_bc[:], ci_col[:], [P, H])
            cj_col = persist.tile([P, 1], f32)
            mul_reduce(junkp, "junk", bj_bc[:], wout_bc[:], cj_col[:], [P, H])

            # const_col = 0.5*(c_i + c_j) + b_out
            const_col = persist.tile([P, 1], f32)
            nc.vector.tensor_add(const_col[:], ci_col[:], cj_col[:])
            nc.vector.tensor_scalar(
                out=const_col[:], in0=const_col[:],
                scalar1=0.5, scalar2=bout_col[:, 0:1],
                op0=mybir.AluOpType.mult, op1=mybir.AluOpType.add,
            )

            # v rows broadcast to all partitions: transpose Vcols then bounce
            vt_ps = psum.tile([4, P], f32, tag="ps_small")
            nc.tensor.transpose(vt_ps[:], vcols[:], ident_sb[:])
            vt_sb = persist.tile([4, P], f32)
            nc.vector.tensor_copy(vt_sb[:], vt_ps[:])
            v_dram = drampool.tile([4, P], f32)
            nc.sync.dma_start(out=v_dram[:], in_=vt_sb[:])
            vi_bc = persist.tile([P, D], f32)
            nc.gpsimd.dma_start(
                out=vi_bc[:],
                in_=bass.AP(tensor=v_dram[:].tensor, offset=v_dram[:].offset,
                            ap=[[0, P], [1, D]]),
            )
            vj_bc = persist.tile([P, D], f32)
            nc.gpsimd.dma_start(
                out=vj_bc[:],
                in_=bass.AP(tensor=v_dram[:].tensor, offset=v_dram[:].offset + D,
                            ap=[[0, P], [1, D]]),
            )

            # ---- load x ----
            xall_sb = persist.tile([P, NBLK_ALL, D], f32)
            xall_r = xall.rearrange("(k p) d -> p k d", p=P)
            for g in range(4):
                nc.sync.dma_start(
                    out=xall_sb[:, g * 8 : (g + 1) * 8, :],
                    in_=xall_r[:, g * 8 : (g + 1) * 8, :],
                )
            xown_sb = persist.tile([P, NBLK_OWN, D], f32)
            xown_r = xown.rearrange("(k p) d -> p k d", p=P)
            for g in range(2):
                nc.sync.dma_start(
                    out=xown_sb[:, g * 8 : (g + 1) * 8, :],
                    in_=xown_r[:, g * 8 : (g + 1) * 8, :],
                )

            # ---- s_j over all blocks, s_i over own blocks ----
            sj_cols = persist.tile([P, NBLK_ALL], f32)
            for k in range(NBLK_ALL):
                mul_reduce(junkp, "junk2", xall_sb[:, k, :], vj_bc[:],
                           sj_cols[:, k : k + 1], [P, D])
            si_cols = persist.tile([P, NBLK_OWN], f32)
            for k in range(NBLK_OWN):
                mul_reduce(junkp, "junk2", xown_sb[:, k, :], vi_bc[:],
                           si_cols[:, k : k + 1], [P, D])

            # bias columns: A = 0.5*s_i + const
            a_cols = persist.tile([P, NBLK_OWN], f32)
            nc.vector.tensor_scalar(
                out=a_cols[:], in0=si_cols[:],
                scalar1=0.5, scalar2=const_col[:, 0:1],
                op0=mybir.AluOpType.mult, op1=mybir.AluOpType.add,
            )

            # s_j row: transpose cols -> (32, 128) -> DRAM (flat 4096) -> bcast
            sjt_ps = psum.tile([NBLK_ALL, P], f32, tag="ps_small")
            nc.tensor.transpose(sjt_ps[:], sj_cols[:], ident_sb[:])
            sjt_sb = persist.tile([NBLK_ALL, P], f32)
            nc.vector.tensor_copy(sjt_sb[:], sjt_ps[:])
            sj_dram = drampool.tile([NBLK_ALL, P], f32)
            nc.sync.dma_start(out=sj_dram[:], in_=sjt_sb[:])
            rb = persist.tile([P, L], f32)
            nc.gpsimd.dma_start(
                out=rb[:],
                in_=bass.AP(tensor=sj_dram[:].tensor, offset=sj_dram[:].offset,
                            ap=[[0, P], [1, L]]),
            )

            # ---- output tiles ----
            for t in range(NBLK_OWN):
                ot = outp.tile([P, L], f32, tag="ot")
                nc.scalar.activation(
                    ot[:], rb[:], mybir.ActivationFunctionType.Relu,
                    bias=a_cols[:, t : t + 1], scale=0.5,
                )
                nc.sync.dma_start(out=out[t * P : (t + 1) * P, :], in_=ot[:])

    nc.finalize()
    return nc


def _get_program():
    global _PROGRAM
    if _PROGRAM is None:
        _PROGRAM = _build_program()
    return _PROGRAM


def _run(inputs, trace=False):
    from concourse.bass_utils import run_bass_kernel_spmd

    x = np.asarray(inputs["x"], np.float32)
    w_i = np.ascontiguousarray(np.asarray(inputs["w_i"], np.float32))
    w_j = np.ascontiguousarray(np.asarray(inputs["w_j"], np.float32))
    b_i = np.asarray(inputs["b_i"], np.float32).reshape(1, H)
    b_j = np.asarray(inputs["b_j"], np.float32).reshape(1, H)
    w_out = np.asarray(inputs["w_out"], np.float32).reshape(1, H)
    b_out = np.asarray(inputs["b_out"], np.float32).reshape(1, 1)
    ident = np.eye(P, dtype=np.float32)

    nc = _get_program()
    in_maps = []
    for c in range(NCORES):
        b, r = divmod(c, 2)
        in_maps.append({
            "xall": np.ascontiguousarray(x[b]),
            "xown": np.ascontiguousarray(x[b, r * ROWS_PER_CORE : (r + 1) * ROWS_PER_CORE]),
            "wi": w_i, "wj": w_j, "bi": b_i, "bj": b_j,
            "wout": w_out, "bout": b_out, "ident": ident,
        })
    res = run_bass_kernel_spmd(nc, in_maps, core_ids=list(range(NCORES)), trace=trace)
    full = np.empty((B, L, L), np.float32)
    for c in range(NCORES):
        b, r = divmod(c, 2)
        full[b, r * ROWS_PER_CORE : (r + 1) * ROWS_PER_CORE, :] = res.results[c]["out"]
    return full, res


def kernel(**inputs):
    full, _ = _run(inputs, trace=False)
    return full


# revision 3
# speedup vs baseline: 1.1526x; 1.1526x over previous
"""DistogramHead Trainium2 kernel.

Computes out[b, i, j] = relu(0.5*(s_i[b,i] + s_j[b,j]) + b_out) where
  s_i = (x @ w_i + b_i) @ w_out  = x @ v_i + c_i,   v_i = w_i @ w_out
  s_j = (x @ w_j + b_j) @ w_out  = x @ v_j + c_j    (exact linear fold)

Shapes: x (4, 4096, 256) f32 -> out (4, 4096, 4096) f32 (256 MB).
Memory-bound on the output write (32 MB per core at ~358 GB/s HBM).

Sharding over 8 cores: core c handles batch b = c//2, row half r = c%2,
producing the contiguous slab out[b, r*2048:(r+1)*2048, :] (32 MB/core).
Each core receives x[b] transposed (xallT, for s_j over all 4096 columns)
plus its own 2048 rows transposed (xownT, for s_i) - static program, no
per-core control flow, one SPMD NEFF for all 8 cores.

Per-core pipeline:
  1. v_j, v_i columns via DVE multiply+reduce over w chunks (d on partitions).
  2. s rows via PE matmuls: lhsT = v column (stationary), rhs = xT chunks
     (moving, 512-col chunks, 2 d-chunk accumulation in PSUM). s_j computed
     in left/right halves so the output phase can start early.
  3. Rb (128, 4096) = s_j row broadcast to all partitions via
     gpsimd.partition_broadcast (SBUF->SBUF, no HBM traffic).
  4. bias cols: s_i row -> (16,128) SBUF DMA rearrange -> PE transpose ->
     A = 0.5*s_i + (0.5*(c_i+c_j) + b_out).
  5. 32 half-tiles: ACT relu(0.5*Rb_half + A[:, t]) -> 1 MB DMA store.
"""

import numpy as np

B = 4
L = 4096
D = 256
H = 128
P = 128
NCORES = 8
ROWS_PER_CORE = L // 2          # 2048
NBLK_OWN = ROWS_PER_CORE // P   # 16
HALF = L // 2                   # 2048

_PROGRAM = None


def _build_program():
    import concourse.bacc as bacc
    import concourse.bass as bass
    import concourse.tile as tile
    from concourse import mybir

    f32 = mybir.dt.float32
    nc = bacc.Bacc(None)

    xallT = nc.dram_tensor("xallT", [D, L], f32, kind="ExternalInput")
    xownT = nc.dram_tensor("xownT", [D, ROWS_PER_CORE], f32, kind="ExternalInput")
    wi = nc.dram_tensor("wi", [D, H], f32, kind="ExternalInput")
    wj = nc.dram_tensor("wj", [D, H], f32, kind="ExternalInput")
    bi = nc.dram_tensor("bi", [1, H], f32, kind="ExternalInput")
    bj = nc.dram_tensor("bj", [1, H], f32, kind="ExternalInput")
    wout = nc.dram_tensor("wout", [1, H], f32, kind="ExternalInput")
    bout = nc.dram_tensor("bout", [1, 1], f32, kind="ExternalInput")
    ident = nc.dram_tensor("ident", [P, P], f32, kind="ExternalInput")
    out = nc.dram_tensor("out", [ROWS_PER_CORE, L], f32, kind="ExternalOutput")

    def dram_bcast(ap, nparts):
        """AP reading the same DRAM region once per partition (stride 0)."""
        return bass.AP(tensor=ap.tensor, offset=ap.offset, ap=[[0, nparts]] + list(ap.ap))

    with tile.TileContext(nc) as tc:
        with (
            tc.tile_pool(name="persist", bufs=1) as persist,
            tc.tile_pool(name="junkp", bufs=2) as junkp,
            tc.tile_pool(name="outp", bufs=4) as outp,
            tc.tile_pool(name="psum", bufs=2, space="PSUM") as psum,
        ):
            # ---- x loads (big, HWDGE queue) ----
            # xT halves: (128, 2, 2048) each; partition = d mod 128, c = d chunk
            xtL = persist.tile([P, 2, HALF], f32)
            nc.sync.dma_start(
                out=xtL[:], in_=xallT[:, 0:HALF].rearrange("(c p) l -> p c l", p=P))
            xtR = persist.tile([P, 2, HALF], f32)
            nc.sync.dma_start(
                out=xtR[:], in_=xallT[:, HALF:L].rearrange("(c p) l -> p c l", p=P))

            # ---- small loads (SWDGE queue, parallel with the above) ----
            ident_sb = persist.tile([P, P], f32)
            nc.gpsimd.dma_start(out=ident_sb[:], in_=ident[:, :])
            wout_bc = persist.tile([P, H], f32)
            nc.gpsimd.dma_start(out=wout_bc[:], in_=dram_bcast(wout[0, :], P))
            bi_bc = persist.tile([P, H], f32)
            nc.gpsimd.dma_start(out=bi_bc[:], in_=dram_bcast(bi[0, :], P))
            bj_bc = persist.tile([P, H], f32)
            nc.gpsimd.dma_start(out=bj_bc[:], in_=dram_bcast(bj[0, :], P))
            bout_col = persist.tile([P, 1], f32)
            nc.gpsimd.dma_start(out=bout_col[:], in_=dram_bcast(bout[0, :], P))
            wi_sb = persist.tile([P, 2, H], f32)
            nc.gpsimd.dma_start(out=wi_sb[:], in_=wi.rearrange("(c p) h -> p c h", p=P))
            wj_sb = persist.tile([P, 2, H], f32)
            nc.gpsimd.dma_start(out=wj_sb[:], in_=wj.rearrange("(c p) h -> p c h", p=P))
            xoT = persist.tile([P, 2, ROWS_PER_CORE], f32)
            nc.gpsimd.dma_start(
                out=xoT[:], in_=xownT.rearrange("(c p) l -> p c l", p=P))

            # ---- v columns: vcols[:, c, 0] = v_j chunk c, [:, c, 1] = v_i ----
            vcols = persist.tile([P, 2, 2], f32)
            for c in range(2):
                for slot, w_sb in ((0, wj_sb), (1, wi_sb)):
                    junk = junkp.tile([P, H], f32, tag="junk")
                    nc.vector.tensor_mul(junk[:], w_sb[:, c, :], wout_bc[:])
                    nc.vector.reduce_sum(vcols[:, c, slot : slot + 1], junk[:],
                                         axis=mybir.AxisListType.X)

            # c_i, c_j, const = 0.5*(c_i+c_j)+b_out (replicated per partition)
            ci_col = persist.tile([P, 1], f32)
            junk = junkp.tile([P, H], f32, tag="junk")
            nc.vector.tensor_mul(junk[:], bi_bc[:], wout_bc[:])
            nc.vector.reduce_sum(ci_col[:], junk[:], axis=mybir.AxisListType.X)
            cj_col = persist.tile([P, 1], f32)
            junk = junkp.tile([P, H], f32, tag="junk")
            nc.vector.tensor_mul(junk[:], bj_bc[:], wout_bc[:])
            nc.vector.reduce_sum(cj_col[:], junk[:], axis=mybir.AxisListType.X)
            const_col = persist.tile([P, 1], f32)
            nc.vector.tensor_add(const_col[:], ci_col[:], cj_col[:])
            nc.vector.tensor_scalar(
                out=const_col[:], in0=const_col[:],
                scalar1=0.5, scalar2=bout_col[:, 0:1],
                op0=mybir.AluOpType.mult, op1=mybir.AluOpType.add,
            )

            # ---- s rows via PE: lhsT = v col (stationary), xT moving ----
            sj_row = persist.tile([1, L], f32)
            si_row = persist.tile([1, ROWS_PER_CORE], f32)
            rb = persist.tile([P, L], f32)

            def s_half(xt, vslot, dst_row_ap, nchunks):
                ps = psum.tile([1, 512 * nchunks], f32, tag="ps")
                for n in range(nchunks):
                    for c in range(2):
                        nc.tensor.matmul(
                            ps[:, n * 512 : (n + 1) * 512],
                            vcols[:, c, vslot : vslot + 1],
                            xt[:, c, n * 512 : (n + 1) * 512],
                            start=(c == 0), stop=(c == 1),
                        )
                nc.scalar.copy(dst_row_ap, ps[:])

            # left half of s_j, then broadcast into rb left
            s_half(xtL, 0, sj_row[0:1, 0:HALF], 4)
            nc.gpsimd.partition_broadcast(rb[:, 0:HALF], sj_row[0:1, 0:HALF])
            # s_i (own rows)
            s_half(xoT, 1, si_row[0:1, :], 4)
            # right half of s_j
            s_half(xtR, 0, sj_row[0:1, HALF:L], 4)
            nc.gpsimd.partition_broadcast(rb[:, HALF:L], sj_row[0:1, HALF:L])

            # ---- bias columns: si_row -> (16,128) -> transpose -> A ----
            si16 = persist.tile([NBLK_OWN, P], f32)
            nc.gpsimd.dma_start(out=si16[:], in_=si_row[0:1, :])
            tr_ps = psum.tile([P, NBLK_OWN], f32, tag="ps")
            nc.tensor.transpose(tr_ps[:], si16[:], ident_sb[0:NBLK_OWN, 0:NBLK_OWN])
            a_cols = persist.tile([P, NBLK_OWN], f32)
            nc.vector.tensor_scalar(
                out=a_cols[:], in0=tr_ps[:],
                scalar1=0.5, scalar2=const_col[:, 0:1],
                op0=mybir.AluOpType.mult, op1=mybir.AluOpType.add,
            )

            # ---- output: 32 half tiles ----
            for half in range(2):
                j0 = half * HALF
                for t in range(NBLK_OWN):
                    ot = outp.tile([P, HALF], f32, tag="ot")
                    nc.scalar.activation(
                        ot[:], rb[:, j0 : j0 + HALF],
                        mybir.ActivationFunctionType.Relu,
                        bias=a_cols[:, t : t + 1], scale=0.5,
                    )
                    nc.sync.dma_start(
                        out=out[t * P : (t + 1) * P, j0 : j0 + HALF], in_=ot[:])

    nc.finalize()
    return nc


def _get_program():
    global _PROGRAM
    if _PROGRAM is None:
        _PROGRAM = _build_program()
    return _PROGRAM


def _run(inputs, trace=False):
    from concourse.bass_utils import run_bass_kernel_spmd

    x = np.asarray(inputs["x"], np.float32)
    w_i = np.ascontiguousarray(np.asarray(inputs["w_i"], np.float32))
    w_j = np.ascontiguousarray(np.asarray(inputs["w_j"], np.float32))
    b_i = np.asarray(inputs["b_i"], np.float32).reshape(1, H)
    b_j = np.asarray(inputs["b_j"], np.float32).reshape(1, H)
    w_out = np.asarray(inputs["w_out"], np.float32).reshape(1, H)
    b_out = np.asarray(inputs["b_out"], np.float32).reshape(1, 1)
    ident = np.eye(P, dtype=np.float32)

    xT = [np.ascontiguousarray(x[b].T) for b in range(B)]  # (256, 4096) each

    nc = _get_program()
    in_maps = []
    for c in range(NCORES):
        b, r = divmod(c, 2)
        in_maps.append({
            "xallT": xT[b],
            "xownT": np.ascontiguousarray(xT[b][:, r * ROWS_PER_CORE : (r + 1) * ROWS_PER_CORE]),
            "wi": w_i, "wj": w_j, "bi": b_i, "bj": b_j,
            "wout": w_out, "bout": b_out, "ident": ident,
        })
    res = run_bass_kernel_spmd(nc, in_maps, core_ids=list(range(NCORES)), trace=trace)
    full = np.empty((B, L, L), np.float32)
    for c in range(NCORES):
        b, r = divmod(c, 2)
        full[b, r * ROWS_PER_CORE : (r + 1) * ROWS_PER_CORE, :] = res.results[c]["out"]
    return full, res


def kernel(**inputs):
    full, _ = _run(inputs, trace=False)
    return full


# revision 5
# speedup vs baseline: 1.1956x; 1.0373x over previous
"""DistogramHead Trainium2 kernel.

Computes out[b, i, j] = relu(0.5*(s_i[b,i] + s_j[b,j]) + b_out) where
  s_i = (x @ w_i + b_i) @ w_out  = x @ v_i + c_i,   v_i = w_i @ w_out
  s_j = (x @ w_j + b_j) @ w_out  = x @ v_j + c_j    (exact linear fold)

Shapes: x (4, 4096, 256) f32 -> out (4, 4096, 4096) f32 (256 MB).
Memory-bound on the output write (32 MB per core at ~358 GB/s HBM).

Sharding over 8 cores: core c handles batch b = c//2, row half r = c%2,
producing the contiguous slab out[b, r*2048:(r+1)*2048, :] (32 MB/core).
Each core receives x[b] transposed (xallT) in 512-column chunks plus a
tiny per-core one-hot selection matrix SEL - one static SPMD program, no
per-core control flow.

Per-core pipeline:
  1. v_j, v_i columns via DVE multiply+reduce over w chunks (d on partitions).
  2. s rows via PE matmuls: lhsT = [v_j, v_i] (stationary, M=2), rhs = xT
     512-col chunks (moving), 2 d-chunk accumulation in PSUM. One pass gives
     s_j AND s_i for all 4096 tokens, in two halves for early output start.
  3. Rb (128, 4096) = s_j row broadcast to all partitions via
     gpsimd.partition_broadcast (SBUF->SBUF, no HBM traffic).
  4. bias cols: s_i row -> (32,128) SBUF rearrange DMA -> PE matmul with the
     per-core SEL matrix (transpose + own-row selection in one op) ->
     A = 0.5*s_i_own + (0.5*(c_i+c_j) + b_out).
  5. 32 half-tiles: ACT relu(0.5*Rb_half + A[:, t]) -> 1 MB DMA store.
"""

import numpy as np

B = 4
L = 4096
D = 256
H = 128
P = 128
NCORES = 8
ROWS_PER_CORE = L // 2          # 2048
NBLK_OWN = ROWS_PER_CORE // P   # 16
NBLK_ALL = L // P               # 32
HALF = L // 2                   # 2048
NCHUNK = 8                      # 512-col x chunks

_PROGRAM = None


def _build_program():
    import concourse.bacc as bacc
    import concourse.bass as bass
    import concourse.tile as tile
    from concourse import mybir

    f32 = mybir.dt.float32
    nc = bacc.Bacc(None)

    xallT = nc.dram_tensor("xallT", [D, L], f32, kind="ExternalInput")
    wi = nc.dram_tensor("wi", [D, H], f32, kind="ExternalInput")
    wj = nc.dram_tensor("wj", [D, H], f32, kind="ExternalInput")
    bi = nc.dram_tensor("bi", [1, H], f32, kind="ExternalInput")
    bj = nc.dram_tensor("bj", [1, H], f32, kind="ExternalInput")
    wout = nc.dram_tensor("wout", [1, H], f32, kind="ExternalInput")
    bout = nc.dram_tensor("bout", [1, 1], f32, kind="ExternalInput")
    sel = nc.dram_tensor("sel", [NBLK_ALL, NBLK_OWN], f32, kind="ExternalInput")
    out = nc.dram_tensor("out", [ROWS_PER_CORE, L], f32, kind="ExternalOutput")

    def dram_bcast(ap, nparts):
        """AP reading the same DRAM region once per partition (stride 0)."""
        return bass.AP(tensor=ap.tensor, offset=ap.offset, ap=[[0, nparts]] + list(ap.ap))

    with tile.TileContext(nc) as tc:
        with (
            tc.tile_pool(name="persist", bufs=1) as persist,
            tc.tile_pool(name="junkp", bufs=2) as junkp,
            tc.tile_pool(name="outp", bufs=4) as outp,
            tc.tile_pool(name="psum", bufs=2, space="PSUM") as psum,
        ):
            # ---- x loads: 8 chunks of 512 cols, (128, 2, 512) each (HWDGE) ----
            xts = []
            for n in range(NCHUNK):
                xt = persist.tile([P, 2, 512], f32, tag=f"xt{n}")
                nc.sync.dma_start(
                    out=xt[:],
                    in_=xallT[:, n * 512 : (n + 1) * 512].rearrange(
                        "(c p) l -> p c l", p=P),
                )
                xts.append(xt)

            # ---- small loads (SWDGE queue, parallel) ----
            wout_bc = persist.tile([P, H], f32)
            nc.gpsimd.dma_start(out=wout_bc[:], in_=dram_bcast(wout[0, :], P))
            bi_bc = persist.tile([P, H], f32)
            nc.gpsimd.dma_start(out=bi_bc[:], in_=dram_bcast(bi[0, :], P))
            bj_bc = persist.tile([P, H], f32)
            nc.gpsimd.dma_start(out=bj_bc[:], in_=dram_bcast(bj[0, :], P))
            bout_col = persist.tile([P, 1], f32)
            nc.gpsimd.dma_start(out=bout_col[:], in_=dram_bcast(bout[0, :], P))
            wi_sb = persist.tile([P, 2, H], f32)
            nc.gpsimd.dma_start(out=wi_sb[:], in_=wi.rearrange("(c p) h -> p c h", p=P))
            wj_sb = persist.tile([P, 2, H], f32)
            nc.gpsimd.dma_start(out=wj_sb[:], in_=wj.rearrange("(c p) h -> p c h", p=P))
            sel_sb = persist.tile([NBLK_ALL, NBLK_OWN], f32)
            nc.gpsimd.dma_start(out=sel_sb[:], in_=sel[:, :])

            # ---- v columns: vcols[:, c, 0] = v_j chunk c, [:, c, 1] = v_i ----
            vcols = persist.tile([P, 2, 2], f32)
            for c in range(2):
                for slot, w_sb in ((0, wj_sb), (1, wi_sb)):
                    junk = junkp.tile([P, H], f32, tag="junk")
                    nc.vector.tensor_mul(junk[:], w_sb[:, c, :], wout_bc[:])
                    nc.vector.reduce_sum(vcols[:, c, slot : slot + 1], junk[:],
                                         axis=mybir.AxisListType.X)

            # c_i, c_j, const = 0.5*(c_i+c_j)+b_out (replicated per partition)
            ci_col = persist.tile([P, 1], f32)
            junk = junkp.tile([P, H], f32, tag="junk")
            nc.vector.tensor_mul(junk[:], bi_bc[:], wout_bc[:])
            nc.vector.reduce_sum(ci_col[:], junk[:], axis=mybir.AxisListType.X)
            cj_col = persist.tile([P, 1], f32)
            junk = junkp.tile([P, H], f32, tag="junk")
            nc.vector.tensor_mul(junk[:], bj_bc[:], wout_bc[:])
            nc.vector.reduce_sum(cj_col[:], junk[:], axis=mybir.AxisListType.X)
            const_col = persist.tile([P, 1], f32)
            nc.vector.tensor_add(const_col[:], ci_col[:], cj_col[:])
            nc.vector.tensor_scalar(
                out=const_col[:], in0=const_col[:],
                scalar1=0.5, scalar2=bout_col[:, 0:1],
                op0=mybir.AluOpType.mult, op1=mybir.AluOpType.add,
            )

            # ---- s rows via PE: lhsT = [v_j, v_i] (stationary), xT moving ----
            # rows_sb row 0 = s_j (all 4096), row 1 = s_i (all 4096)
            rows_sb = persist.tile([2, L], f32)
            rb = persist.tile([P, L], f32)

            for half in range(2):
                ps = psum.tile([2, HALF], f32, tag="ps")
                for n in range(4):
                    for c in range(2):
                        nc.tensor.matmul(
                            ps[:, n * 512 : (n + 1) * 512],
                            vcols[:, c, :],
                            xts[half * 4 + n][:, c, :],
                            start=(c == 0), stop=(c == 1),
                        )
                j0 = half * HALF
                nc.scalar.copy(rows_sb[0:2, j0 : j0 + HALF], ps[:])
                nc.gpsimd.partition_broadcast(
                    rb[:, j0 : j0 + HALF], rows_sb[0:1, j0 : j0 + HALF])

            # ---- bias cols: si row -> (32,128) -> SEL matmul -> A ----
            si32 = persist.tile([NBLK_ALL, P], f32)
            nc.gpsimd.dma_start(out=si32[:], in_=rows_sb[1:2, :])
            asel_ps = psum.tile([P, NBLK_OWN], f32, tag="ps")
            nc.tensor.matmul(asel_ps[:], si32[:], sel_sb[:])
            a_cols = persist.tile([P, NBLK_OWN], f32)
            nc.vector.tensor_scalar(
                out=a_cols[:], in0=asel_ps[:],
                scalar1=0.5, scalar2=const_col[:, 0:1],
                op0=mybir.AluOpType.mult, op1=mybir.AluOpType.add,
            )

            # ---- output: 32 half tiles ----
            for half in range(2):
                j0 = half * HALF
                for t in range(NBLK_OWN):
                    ot = outp.tile([P, HALF], f32, tag="ot")
                    nc.scalar.activation(
                        ot[:], rb[:, j0 : j0 + HALF],
                        mybir.ActivationFunctionType.Relu,
                        bias=a_cols[:, t : t + 1], scale=0.5,
                    )
                    nc.sync.dma_start(
                        out=out[t * P : (t + 1) * P, j0 : j0 + HALF], in_=ot[:])

    nc.finalize()
    return nc


def _get_program():
    global _PROGRAM
    if _PROGRAM is None:
        _PROGRAM = _build_program()
    return _PROGRAM


def _run(inputs, trace=False):
    from concourse.bass_utils import run_bass_kernel_spmd

    x = np.asarray(inputs["x"], np.float32)
    w_i = np.ascontiguousarray(np.asarray(inputs["w_i"], np.float32))
    w_j = np.ascontiguousarray(np.asarray(inputs["w_j"], np.float32))
    b_i = np.asarray(inputs["b_i"], np.float32).reshape(1, H)
    b_j = np.asarray(inputs["b_j"], np.float32).reshape(1, H)
    w_out = np.asarray(inputs["w_out"], np.float32).reshape(1, H)
    b_out = np.asarray(inputs["b_out"], np.float32).reshape(1, 1)

    xT = [np.ascontiguousarray(x[b].T) for b in range(B)]  # (256, 4096) each
    sels = []
    for r in range(2):
        s = np.zeros((NBLK_ALL, NBLK_OWN), np.float32)
        for t in range(NBLK_OWN):
            s[r * NBLK_OWN + t, t] = 1.0
        sels.append(s)

    nc = _get_program()
    in_maps = []
    for c in range(NCORES):
        b, r = divmod(c, 2)
        in_maps.append({
            "xallT": xT[b], "sel": sels[r],
            "wi": w_i, "wj": w_j, "bi": b_i, "bj": b_j,
            "wout": w_out, "bout": b_out,
        })
    res = run_bass_kernel_spmd(nc, in_maps, core_ids=list(range(NCORES)), trace=trace)
    full = np.empty((B, L, L), np.float32)
    for c in range(NCORES):
        b, r = divmod(c, 2)
        full[b, r * ROWS_PER_CORE : (r + 1) * ROWS_PER_CORE, :] = res.results[c]["out"]
    return full, res


def kernel(**inputs):
    full, _ = _run(inputs, trace=False)
    return full


# revision 6
# speedup vs baseline: 1.4436x; 1.2074x over previous
"""DistogramHead Trainium2 kernel.

Computes out[b, i, j] = relu(0.5*(s_i[b,i] + s_j[b,j]) + b_out) where
  s_i = (x @ w_i + b_i) @ w_out  = x @ v_i + c_i,   v_i = w_i @ w_out
  s_j = (x @ w_j + b_j) @ w_out  = x @ v_j + c_j    (exact linear fold)

Shapes: x (4, 4096, 256) f32 -> out (4, 4096, 4096) f32 (256 MB).
Memory-bound on the output write (32 MB per core at ~358 GB/s HBM).

Sharding over 8 cores: core c handles batch b = c//2, row half r = c%2,
producing the contiguous slab out[b, r*2048:(r+1)*2048, :] (32 MB/core).
Each core receives x[b] transposed (xallT) in 512-column chunks plus a
tiny per-core one-hot selection matrix SEL - one static SPMD program, no
per-core control flow.

Per-core pipeline:
  1. v_j, v_i columns via DVE multiply+reduce over w chunks (d on partitions).
  2. s rows via PE matmuls: lhsT = [v_j, v_i] (stationary, M=2), rhs = xT
     512-col chunks (moving), 2 d-chunk accumulation in PSUM. One pass gives
     s_j AND s_i for all 4096 tokens, in two halves for early output start.
  3. Rb (128, 4096) = s_j row broadcast to all partitions via
     gpsimd.partition_broadcast (SBUF->SBUF, no HBM traffic).
  4. bias cols: s_i row -> (32,128) SBUF rearrange DMA -> PE matmul with the
     per-core SEL matrix (transpose + own-row selection in one op) ->
     A = 0.5*s_i_own + (0.5*(c_i+c_j) + b_out).
  5. 32 half-tiles: ACT relu(0.5*Rb_half + A[:, t]) -> 1 MB DMA store.
"""

import numpy as np

B = 4
L = 4096
D = 256
H = 128
P = 128
NCORES = 8
ROWS_PER_CORE = L // 2          # 2048
NBLK_OWN = ROWS_PER_CORE // P   # 16
NBLK_ALL = L // P               # 32
HALF = L // 2                   # 2048
NCHUNK = 8                      # 512-col x chunks

_PROGRAM = None


def _build_program():
    import concourse.bacc as bacc
    import concourse.bass as bass
    import concourse.tile as tile
    from concourse import mybir

    f32 = mybir.dt.float32
    nc = bacc.Bacc(None)

    xc = nc.dram_tensor("xc", [P, NCHUNK, 2, 512], f32, kind="ExternalInput")
    wi = nc.dram_tensor("wi", [D, H], f32, kind="ExternalInput")
    wj = nc.dram_tensor("wj", [D, H], f32, kind="ExternalInput")
    bi = nc.dram_tensor("bi", [1, H], f32, kind="ExternalInput")
    bj = nc.dram_tensor("bj", [1, H], f32, kind="ExternalInput")
    wout = nc.dram_tensor("wout", [1, H], f32, kind="ExternalInput")
    bout = nc.dram_tensor("bout", [1, 1], f32, kind="ExternalInput")
    sel = nc.dram_tensor("sel", [NBLK_OWN, 2, NBLK_OWN], f32, kind="ExternalInput")
    out = nc.dram_tensor("out", [ROWS_PER_CORE, L], f32, kind="ExternalOutput")

    def dram_bcast(ap, nparts):
        """AP reading the same DRAM region once per partition (stride 0)."""
        return bass.AP(tensor=ap.tensor, offset=ap.offset, ap=[[0, nparts]] + list(ap.ap))

    with tile.TileContext(nc) as tc:
        with (
            tc.tile_pool(name="persist", bufs=1) as persist,
            tc.tile_pool(name="junkp", bufs=2) as junkp,
            tc.tile_pool(name="outp", bufs=4) as outp,
            tc.tile_pool(name="psum", bufs=2, space="PSUM") as psum,
        ):
            # ---- regular small loads first (HWDGE, complete fast) ----
            wi_sb = persist.tile([P, 2, H], f32)
            nc.sync.dma_start(out=wi_sb[:], in_=wi.rearrange("(c p) h -> p c h", p=P))
            wj_sb = persist.tile([P, 2, H], f32)
            nc.sync.dma_start(out=wj_sb[:], in_=wj.rearrange("(c p) h -> p c h", p=P))
            sel_sb = persist.tile([NBLK_OWN, 2, NBLK_OWN], f32)
            nc.sync.dma_start(out=sel_sb[:], in_=sel[:, :, :])

            # ---- x loads: 8 chunks, 4 KB/partition contiguous descriptors ----
            xts = []
            for n in range(NCHUNK):
                xt = persist.tile([P, 2, 512], f32, tag=f"xt{n}")
                nc.sync.dma_start(out=xt[:], in_=xc[:, n, :, :])
                xts.append(xt)

            # ---- small loads (SWDGE queue, parallel) ----
            wout_bc = persist.tile([P, H], f32)
            nc.gpsimd.dma_start(out=wout_bc[:], in_=dram_bcast(wout[0, :], P))
            bi_bc = persist.tile([P, H], f32)
            nc.gpsimd.dma_start(out=bi_bc[:], in_=dram_bcast(bi[0, :], P))
            bj_bc = persist.tile([P, H], f32)
            nc.gpsimd.dma_start(out=bj_bc[:], in_=dram_bcast(bj[0, :], P))
            bout_col = persist.tile([P, 1], f32)
            nc.gpsimd.dma_start(out=bout_col[:], in_=dram_bcast(bout[0, :], P))

            # ---- v columns: vcols[:, c, 0] = v_j chunk c, [:, c, 1] = v_i ----
            vcols = persist.tile([P, 2, 2], f32)
            for c in range(2):
                for slot, w_sb in ((0, wj_sb), (1, wi_sb)):
                    junk = junkp.tile([P, H], f32, tag="junk")
                    nc.vector.tensor_mul(junk[:], w_sb[:, c, :], wout_bc[:])
                    nc.vector.reduce_sum(vcols[:, c, slot : slot + 1], junk[:],
                                         axis=mybir.AxisListType.X)

            # c_i, c_j, const = 0.5*(c_i+c_j)+b_out (replicated per partition)
            ci_col = persist.tile([P, 1], f32)
            junk = junkp.tile([P, H], f32, tag="junk")
            nc.vector.tensor_mul(junk[:], bi_bc[:], wout_bc[:])
            nc.vector.reduce_sum(ci_col[:], junk[:], axis=mybir.AxisListType.X)
            cj_col = persist.tile([P, 1], f32)
            junk = junkp.tile([P, H], f32, tag="junk")
            nc.vector.tensor_mul(junk[:], bj_bc[:], wout_bc[:])
            nc.vector.reduce_sum(cj_col[:], junk[:], axis=mybir.AxisListType.X)
            const_col = persist.tile([P, 1], f32)
            nc.vector.tensor_add(const_col[:], ci_col[:], cj_col[:])
            nc.vector.tensor_scalar(
                out=const_col[:], in0=const_col[:],
                scalar1=0.5, scalar2=bout_col[:, 0:1],
                op0=mybir.AluOpType.mult, op1=mybir.AluOpType.add,
            )

            # ---- s rows via PE: lhsT = [v_j, v_i] (stationary), xT moving ----
            # rows_sb row 0 = s_j (all 4096), row 1 = s_i (all 4096)
            rows_sb = persist.tile([2, L], f32)
            rb = persist.tile([P, L], f32)

            for half in range(2):
                ps = psum.tile([2, HALF], f32, tag="ps")
                for n in range(4):
                    for c in range(2):
                        nc.tensor.matmul(
                            ps[:, n * 512 : (n + 1) * 512],
                            vcols[:, c, :],
                            xts[half * 4 + n][:, c, :],
                            start=(c == 0), stop=(c == 1),
                        )
                j0 = half * HALF
                nc.scalar.copy(rows_sb[0:2, j0 : j0 + HALF], ps[:])
                nc.gpsimd.partition_broadcast(
                    rb[:, j0 : j0 + HALF], rows_sb[0:1, j0 : j0 + HALF])

            # ---- bias cols: si halves -> (16,128) -> SEL matmuls -> A ----
            asel_ps = psum.tile([P, NBLK_OWN], f32, tag="ps")
            si16s = []
            for half in range(2):
                s16 = persist.tile([NBLK_OWN, P], f32, tag=f"si16_{half}")
                si16s.append(s16)
            for half in range(2):
                nc.sync.dma_start(
                    out=si16s[half][:],
                    in_=rows_sb[1:2, half * HALF : (half + 1) * HALF])
                nc.tensor.matmul(asel_ps[:], si16s[half][:], sel_sb[:, half, :],
                                 start=(half == 0), stop=(half == 1))
            a_cols = persist.tile([P, NBLK_OWN], f32)
            nc.vector.tensor_scalar(
                out=a_cols[:], in0=asel_ps[:],
                scalar1=0.5, scalar2=const_col[:, 0:1],
                op0=mybir.AluOpType.mult, op1=mybir.AluOpType.add,
            )

            # ---- output: 32 half tiles ----
            for half in range(2):
                j0 = half * HALF
                for t in range(NBLK_OWN):
                    ot = outp.tile([P, HALF], f32, tag="ot")
                    nc.scalar.activation(
                        ot[:], rb[:, j0 : j0 + HALF],
                        mybir.ActivationFunctionType.Relu,
                        bias=a_cols[:, t : t + 1], scale=0.5,
                    )
                    nc.sync.dma_start(
                        out=out[t * P : (t + 1) * P, j0 : j0 + HALF], in_=ot[:])

    nc.finalize()
    return nc


def _get_program():
    global _PROGRAM
    if _PROGRAM is None:
        _PROGRAM = _build_program()
    return _PROGRAM


def _run(inputs, trace=False):
    from concourse.bass_utils import run_bass_kernel_spmd

    x = np.asarray(inputs["x"], np.float32)
    w_i = np.ascontiguousarray(np.asarray(inputs["w_i"], np.float32))
    w_j = np.ascontiguousarray(np.asarray(inputs["w_j"], np.float32))
    b_i = np.asarray(inputs["b_i"], np.float32).reshape(1, H)
    b_j = np.asarray(inputs["b_j"], np.float32).reshape(1, H)
    w_out = np.asarray(inputs["w_out"], np.float32).reshape(1, H)
    b_out = np.asarray(inputs["b_out"], np.float32).reshape(1, 1)

    # pre-chunked xT: xcs[b][p, n, c, l] = x[b][n*512+l, c*128+p]
    xcs = [np.ascontiguousarray(
        x[b].T.reshape(2, P, NCHUNK, 512).transpose(1, 2, 0, 3)) for b in range(B)]
    eye = np.eye(NBLK_OWN, dtype=np.float32)
    sels = []
    for r in range(2):
        s = np.zeros((NBLK_OWN, 2, NBLK_OWN), np.float32)
        s[:, r, :] = eye
        sels.append(s)

    nc = _get_program()
    in_maps = []
    for c in range(NCORES):
        b, r = divmod(c, 2)
        in_maps.append({
            "xc": xcs[b], "sel": sels[r],
            "wi": w_i, "wj": w_j, "bi": b_i, "bj": b_j,
            "wout": w_out, "bout": b_out,
        })
    res = run_bass_kernel_spmd(nc, in_maps, core_ids=list(range(NCORES)), trace=trace)
    full = np.empty((B, L, L), np.float32)
    for c in range(NCORES):
        b, r = divmod(c, 2)
        full[b, r * ROWS_PER_CORE : (r + 1) * ROWS_PER_CORE, :] = res.results[c]["out"]
    return full, res


def kernel(**inputs):
    full, _ = _run(inputs, trace=False)
    return full
